# revision 35
# baseline (speedup 1.0000x reference)
"""Trainium2 Bass kernel for a dense transformer block (pre-LN, causal MHA + FFN).

Sharding: pure data-parallel over batch — 8 sequences -> 8 NeuronCores, no
collectives. Each core runs the full block on its [2048, 400] slice.

Per-core recipe (bf16 matmuls, f32 PSUM/residual/softmax-stats):
  LN gamma/beta are folded into the consuming weights host-side
  (Wq' = diag(g)Wq etc., bias' = beta@W), so device LN is just
  z = (x - mu) * rstd -> bf16, with rstd via DVE Newton (no ACT Sqrt,
  avoids activation-table switches between Sqrt and Exp sets).
  qT   = bf16(0.1*Wq[h].T @ zT + bq)   [100(d), 2048] per head
  kT   = bf16(Wk[h].T @ zT + bk)
  v1   = bf16(zT.T @ Wv_all + bv) rows + ones column  [2048(s), H, 102]
  attention per head over t-tiles of 512: scoresT = kT_chunk.T @ qT_tile
  pairs of [s128, t512] score tiles land in one 2-bank PSUM tile and are
  Exp'd by a single ACT instr ([128,1024], amortizes the 352-cycle ACT
  startup); causal mask added on diagonal pairs; diagonal groups are
  trimmed to the causally-needed columns. probsT bf16 goes directly into
  attn@V lhsT layout; attn@V accumulates [t128, 102] with a softmax-
  denominator ones column; rows scaled by 1/denom at copy-out; transposed
  once into attn_oT [100(d), head, 2048].
  proj = sum_h attn_oT[h].T @ Wo[h] + residual into x (f32)
  LN2 -> z2T (reuses zT slot); FFN pipelined in 512-column slices:
  ffT = relu(W1.T @ z2T + b1') bf16, then fc2 rows = ffT.T @ W2 +
  residual + b2 -> out (f32).

All weight reshaping/casting/LN-folding is host-side numpy.
"""

import numpy as np
import ml_dtypes

import concourse.bass as bass
import concourse.mybir as mybir
import concourse.tile as tile
from concourse import bacc
from concourse.bass_utils import run_bass_kernel_spmd

BF16NP = ml_dtypes.bfloat16
BF16 = mybir.dt.bfloat16
F32 = mybir.dt.float32
AF = mybir.ActivationFunctionType
ALU = mybir.AluOpType

P = 128          # partitions
B = 8            # batch -> cores
T = 2048         # sequence length
C = 400          # embed dim
H = 4            # heads
D = 100          # head dim
DFF = 1600       # ffn hidden
NT = T // P      # 16 row tiles
NCC = C // D     # 4 contraction chunks of 100
WT = 512         # wide tile for qkv matmuls
NWT = T // WT    # 4
TJ = 512         # t-tile width for transposed attention scores
NTJ = T // TJ    # 4
SUB = TJ // P    # 4 t128 sub-blocks per score tile
FT = 512         # ffn column-slice width
NFT = T // FT    # 4
NFC = (DFF + P - 1) // P  # 13 f-chunks (12x128 + 64)
NEG = -1.0e30
EXP_GROUP = 1    # score tiles per Exp instruction (1 or 2)

LAST_RESULT = None  # BassKernelResults of the most recent run (for test.py)


def _fchunk(fc):
    return min(P, DFF - fc * P)


def build_block(loop_n=None, phases=("qkv", "attn", "proj", "ffn")):
    nc = bacc.Bacc("TRN2", target_bir_lowering=False, debug=False)

    x_d = nc.dram_tensor("x", [T, C], F32, kind="ExternalInput")
    wq_d = nc.dram_tensor("wqp", [D + 1, H, NCC, P], BF16,
                          kind="ExternalInput")
    wk_d = nc.dram_tensor("wkp", [D + 1, H, NCC, P], BF16,
                          kind="ExternalInput")
    wv_d = nc.dram_tensor("wvp", [D + 1, NCC, C], BF16, kind="ExternalInput")
    wo_d = nc.dram_tensor("wop", [D, H, C], BF16, kind="ExternalInput")
    w1_d = nc.dram_tensor("w1p", [D, NCC, DFF], BF16, kind="ExternalInput")
    w2_d = nc.dram_tensor("w2p", [P, NFC, C], BF16, kind="ExternalInput")
    b1_d = nc.dram_tensor("b1p", [P, NFC], F32, kind="ExternalInput")
    bo_d = nc.dram_tensor("bop", [P, C], F32, kind="ExternalInput")
    b2_d = nc.dram_tensor("b2p", [P, C], F32, kind="ExternalInput")
    mask_d = nc.dram_tensor("maskp", [P, P], F32, kind="ExternalInput")
    id_d = nc.dram_tensor("identp", [P, P], BF16, kind="ExternalInput")
    out_d = nc.dram_tensor("out", [T, C], F32, kind="ExternalOutput")

    with tile.TileContext(nc) as tc:
        with (
            tc.tile_pool(name="consts", bufs=1) as consts,
            tc.tile_pool(name="persist", bufs=1) as persist,
            tc.tile_pool(name="qk", bufs=2) as qk_pool,
            tc.tile_pool(name="pr", bufs=2) as pr_pool,
            tc.tile_pool(name="fft", bufs=2) as fft_pool,
            tc.tile_pool(name="work", bufs=3) as work,
            tc.tile_pool(name="small", bufs=4) as small,
            tc.tile_pool(name="ps_mm", bufs=3, space="PSUM") as ps_mm,
            tc.tile_pool(name="ps_av", bufs=2, space="PSUM") as ps_av,
            tc.tile_pool(name="ps_tr", bufs=1, space="PSUM") as ps_tr,
            tc.tile_pool(name="ps_g", bufs=2, space="PSUM") as ps_g,
        ):
            # ---- constants into SBUF (one-time) ----
            def cload(tag, dram, shape, dtype, psz=P):
                t_ = consts.tile(shape, dtype, tag=tag)
                nc.sync.dma_start(t_[:psz], dram[:])
                return t_

            wq_sb = cload("wq", wq_d, [P, H, NCC, P], BF16, D + 1)
            wk_sb = cload("wk", wk_d, [P, H, NCC, P], BF16, D + 1)
            wv_sb = cload("wv", wv_d, [P, NCC, C], BF16, D + 1)
            wo_sb = cload("wo", wo_d, [P, H, C], BF16, D)
            w1_sb = cload("w1", w1_d, [P, NCC, DFF], BF16, D)
            w2_sb = cload("w2", w2_d, [P, NFC, C], BF16)
            b1_sb = cload("b1", b1_d, [P, NFC], F32)
            bo_sb = cload("bo", bo_d, [P, C], F32)
            b2_sb = cload("b2", b2_d, [P, C], F32)
            mask_sb = cload("mask", mask_d, [P, P], F32)
            id_sb = cload("ident", id_d, [P, P], BF16)

            def trtile4():
                """[P, 1024] bf16 view of a one-bank PSUM tile; holds FOUR
                [P,128] transpose results so one wide copy evacuates them."""
                t_ = ps_tr.tile([P, WT], F32, tag="tr")
                return t_.bitcast(BF16)

            def body():
                x_sb = persist.tile([P, NT, C], F32, tag="x")
                for ti in range(NT):
                    nc.sync.dma_start(x_sb[:, ti, :],
                                      x_d[ti * P:(ti + 1) * P, :])

                # hT row D (=100) is a constant-1.0 "bias row": Q/K/V biases
                # live in row D of the cc=0 weight chunks, so the matmul
                # contraction adds them for free. DVE base partitions must be
                # 32-aligned, so set rows 96.. (96-99 are re-written by the
                # LN transposes before any matmul reads them).
                hT_sb = persist.tile([P, NCC, T], BF16, tag="hT")
                nc.vector.memset(hT_sb[96:, :, :], 1.0)
                v1_sb = persist.tile([P, NT, H, D + 2], BF16, tag="v")
                nc.vector.memset(v1_sb[:, :, :, D], 1.0)
                nc.vector.memset(v1_sb[:, :, :, D + 1], 0.0)
                ao_sb = persist.tile([P, H, T], BF16, tag="aoT")

                def rsqrt_newton(dst, var_ap, n):
                    """dst[P,n] = 1/sqrt(var+1e-5), DVE-only Newton from
                    y0 = 1/v (converges for v > 1/3; LN var ~ 1)."""
                    v = small.tile([P, NT], F32, tag="nv")
                    nc.vector.tensor_scalar_add(out=v[:, :n], in0=var_ap,
                                                scalar1=1e-5)
                    nc.vector.reciprocal(out=dst, in_=v[:, :n])
                    t = small.tile([P, NT], F32, tag="ntm")
                    for _ in range(3):
                        nc.vector.tensor_mul(out=t[:, :n], in0=dst, in1=dst)
                        nc.vector.tensor_mul(out=t[:, :n], in0=t[:, :n],
                                             in1=v[:, :n])
                        nc.vector.tensor_scalar(
                            out=t[:, :n], in0=t[:, :n],
                            scalar1=-0.5, scalar2=1.5,
                            op0=ALU.mult, op1=ALU.add)
                        nc.vector.tensor_mul(out=dst, in0=dst, in1=t[:, :n])

                def layernorm(src3, dstT, tis):
                    """z = (row - mu) * rstd -> bf16, transposed into
                    dstT[:D, cc, ti*P:(ti+1)*P]. gamma/beta pre-folded into
                    the consuming weights. Processes `tis` in batches of 8
                    so normalize overlaps the next batch's stats."""
                    for k0 in range(0, len(tis), 8):
                        bts = tis[k0:k0 + 8]
                        n = len(bts)
                        mv = small.tile([P, 8, 2], F32, tag="mv")
                        for k, ti in enumerate(bts):
                            stats = small.tile([P, 6], F32, tag="stats")
                            nc.vector.bn_stats(out=stats, in_=src3[:, ti, :])
                            nc.vector.bn_aggr(out=mv[:, k, :], in_=stats)
                        rstd = small.tile([P, 8], F32, tag="rstd")
                        rsqrt_newton(rstd[:, :n], mv[:, :n, 1], n)
                        for k, ti in enumerate(bts):
                            hbf = work.tile([P, C], BF16, tag="hbf")
                            nc.vector.tensor_scalar(
                                out=hbf, in0=src3[:, ti, :],
                                scalar1=mv[:, k, 0:1],
                                scalar2=rstd[:, k:k + 1],
                                op0=ALU.subtract, op1=ALU.mult)
                            ptb = trtile4()
                            for cc in range(NCC):
                                nc.tensor.transpose(
                                    ptb[:D, cc * P:(cc + 1) * P],
                                    hbf[:, cc * D:(cc + 1) * D], id_sb)
                            dst = dstT[:D, :, ti * P:(ti + 1) * P]
                            src = ptb[:D, :NCC * P].rearrange(
                                "p (c t) -> p c t", c=NCC)
                            if ti % 2 == 0:
                                nc.vector.tensor_copy(out=dst, in_=src)
                            else:
                                nc.scalar.copy(out=dst, in_=src)

                # ---- LN1 + transpose for all row tiles ----
                layernorm(x_sb, hT_sb, list(range(NT)))

                # ---- V rows (all heads) + ones column + bias ----
                for ti in range(NT if "qkv" in phases else 0):
                    psv = ps_mm.tile([P, WT], F32, tag="mm")
                    for cc in range(NCC):
                        nc.tensor.matmul(
                            psv[:, :C],
                            lhsT=hT_sb[:D + 1, cc, ti * P:(ti + 1) * P],
                            rhs=wv_sb[:D + 1, cc, :],
                            start=(cc == 0), stop=(cc == NCC - 1))
                    nc.vector.tensor_copy(
                        out=v1_sb[:, ti, :, :D],
                        in_=psv[:, :C].rearrange("p (h d) -> p h d", h=H))

                # ---- per-head attention (transposed-score form) ----
                # attn@V is software-pipelined one score-tile behind the
                # scores/exp producer (carried across heads) so independent
                # matmuls hide the ACT exp latency on the in-order PE queue.
                def emit_attnv(pjT, h_, j):
                    ptb = trtile4()
                    for jj in range(SUB):
                        ti = SUB * j + jj
                        pso = ps_av.tile([P, WT], F32, tag="av")
                        for si in range(ti + 1):
                            nc.tensor.matmul(
                                pso[:, :D + 2],
                                lhsT=pjT[:, si, jj * P:(jj + 1) * P],
                                rhs=v1_sb[:, si, h_, :],
                                start=(si == 0), stop=(si == ti))
                        rec = small.tile([P, 1], F32, tag="rec")
                        nc.vector.reciprocal(out=rec, in_=pso[:, D:D + 1])
                        arow = work.tile([P, D], BF16, tag="arow")
                        nc.vector.tensor_scalar_mul(
                            out=arow, in0=pso[:, :D], scalar1=rec)
                        nc.tensor.transpose(ptb[:D, jj * P:(jj + 1) * P],
                                            arow, id_sb)
                    nc.vector.tensor_copy(
                        out=ao_sb[:D, h_, j * TJ:(j + 1) * TJ],
                        in_=ptb[:D, :TJ])

                pend_av = None
                for h in range(H if "qkv" in phases else 0):
                    qT = qk_pool.tile([P, T], BF16, tag="qT")
                    kT = qk_pool.tile([P, T], BF16, tag="kT")
                    for tt in range(NWT):
                        sl = slice(tt * WT, (tt + 1) * WT)
                        psq = ps_mm.tile([P, WT], F32, tag="mm")
                        for cc in range(NCC):
                            nc.tensor.matmul(
                                psq, lhsT=wq_sb[:D + 1, h, cc, :],
                                rhs=hT_sb[:D + 1, cc, sl],
                                start=(cc == 0), stop=(cc == NCC - 1))
                        psk = ps_mm.tile([P, WT], F32, tag="mm")
                        for cc in range(NCC):
                            nc.tensor.matmul(
                                psk, lhsT=wk_sb[:D + 1, h, cc, :],
                                rhs=hT_sb[:D + 1, cc, sl],
                                start=(cc == 0), stop=(cc == NCC - 1))
                        if tt % 2 == 0:
                            nc.vector.tensor_scalar_mul(
                                out=qT[:D, sl], in0=psq[:D, :], scalar1=0.1)
                            nc.scalar.copy(out=kT[:D, sl], in_=psk[:D, :])
                        else:
                            nc.scalar.mul(out=qT[:D, sl], in_=psq[:D, :],
                                          mul=0.1)
                            nc.vector.tensor_copy(out=kT[:D, sl],
                                                  in_=psk[:D, :])

                    for j in range(NTJ if "attn" in phases else 0):
                        icnt = SUB * j + SUB
                        pjT = pr_pool.tile([P, NT, TJ], BF16, tag="probsT")
                        for i in range(icnt):
                            # diagonal t128 sub-blocks (r >= 0): only cols
                            # >= r*P are causally needed; the diagonal block
                            # itself gets the triangular mask. Fully-masked
                            # regions land in probsT cols attn@V never reads.
                            r = i - SUB * j
                            c0 = r * P if r >= 0 else 0
                            pss = ps_mm.tile([P, WT], F32, tag="mm")
                            nc.tensor.matmul(
                                pss[:, c0:TJ],
                                lhsT=kT[:D, i * P:(i + 1) * P],
                                rhs=qT[:D, j * TJ + c0:(j + 1) * TJ],
                                start=True, stop=True)
                            if r >= 0:
                                nc.vector.tensor_add(
                                    out=pss[:, r * P:(r + 1) * P],
                                    in0=pss[:, r * P:(r + 1) * P],
                                    in1=mask_sb)
                            nc.scalar.activation(
                                out=pjT[:, i, c0:TJ],
                                in_=pss[:, c0:TJ], func=AF.Exp)
                        if pend_av is not None:
                            emit_attnv(*pend_av)
                        pend_av = (pjT, h, j)

                if pend_av is not None:
                    emit_attnv(*pend_av)

                # ---- output projection + residual ----
                for ti in range(NT if "proj" in phases else 0):
                    psp = ps_g.tile([P, WT], F32, tag="g")
                    for h in range(H):
                        nc.tensor.matmul(
                            psp[:, :C], lhsT=ao_sb[:D, h, ti * P:(ti + 1) * P],
                            rhs=wo_sb[:D, h, :],
                            start=(h == 0), stop=(h == H - 1))
                    nc.vector.tensor_add(out=x_sb[:, ti, :],
                                         in0=x_sb[:, ti, :], in1=psp[:, :C])
                    nc.gpsimd.tensor_add(out=x_sb[:, ti, :],
                                         in0=x_sb[:, ti, :], in1=bo_sb)

                # ---- FFN, pipelined in FT-column slices ----
                if "ffn" in phases:
                    # same slot as hT (tag rotation, bufs=1); bias row
                    # survives since LN transposes only write rows :D.
                    h2T = persist.tile([P, NCC, T], BF16, tag="hT")
                    layernorm(x_sb, h2T, list(range(NT)))

                    def emit_fc2(ffT, ft):
                        for tl in range(FT // P):
                            ti = ft * (FT // P) + tl
                            psg = ps_g.tile([P, WT], F32, tag="g")
                            for fc in range(NFC):
                                fsz = _fchunk(fc)
                                nc.tensor.matmul(
                                    psg[:, :C],
                                    lhsT=ffT[:fsz, fc, tl * P:(tl + 1) * P],
                                    rhs=w2_sb[:fsz, fc, :],
                                    start=(fc == 0), stop=(fc == NFC - 1))
                            orow = work.tile([P, C], F32, tag="orow")
                            nc.vector.tensor_add(out=orow, in0=psg[:, :C],
                                                 in1=x_sb[:, ti, :])
                            nc.gpsimd.tensor_add(out=orow, in0=orow,
                                                 in1=b2_sb)
                            nc.sync.dma_start(
                                out_d[ti * P:(ti + 1) * P, :], orow)

                    pend_fc2 = None
                    for ft in range(NFT):
                        sl = slice(ft * FT, (ft + 1) * FT)
                        ffT = fft_pool.tile([P, NFC, FT], BF16, tag="ffT")
                        for fc in range(NFC):
                            fsz = _fchunk(fc)
                            psf = ps_mm.tile([P, WT], F32, tag="mm")
                            for cc in range(NCC):
                                nc.tensor.matmul(
                                    psf[:fsz, :],
                                    lhsT=w1_sb[:D, cc, fc * P:fc * P + fsz],
                                    rhs=h2T[:D, cc, sl],
                                    start=(cc == 0), stop=(cc == NCC - 1))
                            if fc % 2 == 0:
                                nc.vector.tensor_scalar(
                                    out=ffT[:fsz, fc, :], in0=psf[:fsz, :],
                                    scalar1=b1_sb[:fsz, fc:fc + 1],
                                    scalar2=0.0, op0=ALU.add, op1=ALU.max)
                            else:
                                nc.scalar.activation(
                                    out=ffT[:fsz, fc, :], in_=psf[:fsz, :],
                                    func=AF.Relu,
                                    bias=b1_sb[:fsz, fc:fc + 1], scale=1.0)
                        if pend_fc2 is not None:
                            emit_fc2(*pend_fc2)
                        pend_fc2 = (ffT, ft)
                    emit_fc2(*pend_fc2)
                else:
                    zrow = work.tile([P, C], F32, tag="orow")
                    nc.vector.memset(zrow, 0.0)
                    for ti in range(NT):
                        nc.sync.dma_start(out_d[ti * P:(ti + 1) * P, :],
                                          zrow)

            if loop_n is None:
                body()
            else:
                with tc.For_i(0, loop_n, 1):
                    body()

    nc.finalize()
    return nc


def prep_weights(Wq, Wk, Wv, Wo, bo, W1, b1, W2, b2,
                 ln1_g, ln1_b, ln2_g, ln2_b):
    """Host-side reshape/cast into the layouts the device program expects.
    LayerNorm gamma/beta are folded into the consuming weights:
      h = z*g + b with z = (x-mu)*rstd, so  h@W = z@(diag(g)W) + b@W.
    """
    f32 = np.float32
    Wq = np.asarray(Wq, f32); Wk = np.asarray(Wk, f32)
    Wv = np.asarray(Wv, f32); Wo = np.asarray(Wo, f32)
    W1 = np.asarray(W1, f32); W2 = np.asarray(W2, f32)
    g1 = np.asarray(ln1_g, f32); be1 = np.asarray(ln1_b, f32)
    g2 = np.asarray(ln2_g, f32); be2 = np.asarray(ln2_b, f32)

    Wqg = Wq * g1[None, :, None]          # [H, C, D]
    Wkg = Wk * g1[None, :, None]
    Wvg = Wv * g1[None, :, None]
    W1g = W1 * g2[:, None]                # [C, DFF]
    bq = np.einsum("c,hcd->hd", be1, Wq)         # [H, D] (0.1 applied later)
    bk = np.einsum("c,hcd->hd", be1, Wk)         # [H, D]
    bv = np.einsum("c,hcd->hd", be1, Wv).reshape(C)  # [(h d)]
    b1f = np.asarray(b1, f32) + be2 @ W1         # [DFF]

    # [H, C, D] -> [c(100)+bias row, H, cc, D->padded 128]
    # row D of the cc=0 chunk carries the LN-beta-induced bias; the hT
    # bias row (constant 1.0) dots with it inside the matmul.
    wqp = np.zeros((D + 1, H, NCC, P), BF16NP)
    wkp = np.zeros((D + 1, H, NCC, P), BF16NP)
    wqp[:D, :, :, :D] = Wqg.reshape(H, NCC, D, D).transpose(2, 0, 1, 3
                                                            ).astype(BF16NP)
    wkp[:D, :, :, :D] = Wkg.reshape(H, NCC, D, D).transpose(2, 0, 1, 3
                                                            ).astype(BF16NP)
    wqp[D, :, 0, :D] = bq.astype(BF16NP)
    wkp[D, :, 0, :D] = bk.astype(BF16NP)
    # [H, C, D] -> [c(100)+bias row, cc, H*D]
    wvp = np.zeros((D + 1, NCC, C), BF16NP)
    wvp[:D] = (Wvg.reshape(H, NCC, D, D).transpose(2, 1, 0, 3)
               .reshape(D, NCC, C).astype(BF16NP))
    wvp[D, 0, :] = bv.astype(BF16NP)
    # [C, C] -> [c_in_head(100), H, C]
    wop = Wo.reshape(H, D, C).transpose(1, 0, 2).astype(BF16NP).copy()
    # [C, DFF] -> [c(100), cc, DFF]
    w1p = W1g.reshape(NCC, D, DFF).transpose(1, 0, 2).astype(BF16NP).copy()
    # [DFF, C] -> [f_in_chunk(128), fc(13), C], zero-padded
    w2p = np.zeros((P, NFC, C), BF16NP)
    b1p = np.zeros((P, NFC), np.float32)
    for fc in range(NFC):
        fsz = _fchunk(fc)
        w2p[:fsz, fc, :] = W2[fc * P:fc * P + fsz, :].astype(BF16NP)
        b1p[:fsz, fc] = b1f[fc * P:fc * P + fsz]
    tilep = lambda a: np.tile(np.asarray(a, f32).reshape(1, C), (P, 1)).copy()
    # transposed-score causal mask [s_local(128), t_local(128)]:
    # 0 where t >= s, NEG where t < s (strict lower triangle masked).
    sl_ = np.arange(P)[:, None]
    tl_ = np.arange(P)[None, :]
    maskp = np.where(tl_ >= sl_, 0.0, NEG).astype(f32)
    ident = np.eye(P, dtype=BF16NP)
    return {
        "wqp": wqp, "wkp": wkp, "wvp": wvp, "wop": wop, "w1p": w1p,
        "w2p": w2p, "b1p": b1p, "bop": tilep(bo), "b2p": tilep(b2),
        "maskp": np.ascontiguousarray(maskp), "identp": ident,
    }


_CACHED_NC = None


def kernel(x, ln1_g, ln1_b, ln2_g, ln2_b, Wq, Wk, Wv, Wo, bo, W1, b1, W2, b2,
           trace=False):
    global _CACHED_NC, LAST_RESULT
    x = np.asarray(x, np.float32)
    assert x.shape == (B, T, C), x.shape
    wmap = prep_weights(Wq, Wk, Wv, Wo, bo, W1, b1, W2, b2,
                        ln1_g, ln1_b, ln2_g, ln2_b)
    if _CACHED_NC is None:
        _CACHED_NC = build_block()
    nc = _CACHED_NC
    in_maps = [dict(wmap, x=np.ascontiguousarray(x[c])) for c in range(B)]
    res = run_bass_kernel_spmd(nc, in_maps, core_ids=list(range(B)),
                               trace=trace)
    LAST_RESULT = res
    out = np.stack([res.results[c]["out"] for c in range(B)])
    return out.astype(np.float32)
